# revision 2
# baseline (speedup 1.0000x reference)
"""TRN2 Bass kernel for nn_Attention_56281251447235.

Multi-head attention: x:[4,2048,1024], w_qkv:[1024,3072] (q|k|v),
16 heads x 64 dim_head, w_out:[1024,1024], b_out:[1024].

Sharding over 8 NeuronCores: core j handles batch b=j//2 and head-group
hg=j%2 (8 of 16 heads).  Each core computes its 8 heads' attention and a
partial output projection [2048,1024] in bf16; the host sums the two
partials per batch in f32 and adds the bias.

All matmul operands bf16 (1 cycle/row at any free size).  Per-core
pipeline:
  P0  DMA wq0/wk0 + xT (kc-chunked) + remaining weights; pair-0 q/k
      projection kc-outer across an 8-bank PSUM pool so the last xT
      chunk is immediately followed by the last accumulation pass.
  P1  per pair p, per i-block ib (1024 wide; the very last head's ib1
      is split 2x512 to shrink the end-of-kernel dependency tail), per
      head h2:
        per jc (16 j-chunks of 128 tokens):
          S^T[j,i-block] = k_h j-chunk @ q_h  (PSUM, 2 matmuls of 512)
          ex = exp(S^T/8) on ACT -> SBUF bf16   (the pacing engine)
          PV: per 128-i chunk: o_acc[i,65] += ex_slice^T @ v_aug
              (v_aug has a ones column so col 64 accumulates the
              softmax denominator; M=i layout keeps the PE at
              128 outputs/cycle instead of 65/128 partitions)
        norm: DVE reciprocal of o_acc[:,:,64] + one broadcast multiply
              into O_pair[:,:,h2,:] bf16
      after both heads of (p, ib): 8 DMA-engine transposes
      [128i x (2x64)d] -> OT[:, p, i-slice]  (XBAR dma transpose)
  P2  output projection dripped into PE slack as OT columns become
      available; partial [2048,1024] bf16 DMA'd out per [128,512] tile.

Dripping: v-projection, pair p+1 q/k projection, and output-projection
units are generators yielding one PE matmul per step, pumped into the
attention loop's PE slack (ACT paces the inner loop at ~1038ns/jc vs
~644ns of PE work).
"""

import numpy as np
import ml_dtypes

import concourse.mybir as mybir
import concourse.tile as tile
from concourse import bacc
from concourse.bass_utils import run_bass_kernel_spmd

F32 = mybir.dt.float32
BF16 = mybir.dt.bfloat16
EXP = mybir.ActivationFunctionType.Exp
bfloat16 = ml_dtypes.bfloat16

P = 128
B, N, DIM = 4, 2048, 1024
H_LOC = 8  # heads per core
D = 64
FEAT = H_LOC * D  # 512
KC = DIM // P  # 8 contraction chunks
NT = N // P  # 16 token chunks (j-chunks)
SCALE = 1.0 / 8.0

_CACHE = {}


def _emit(nc, tc):
    from contextlib import ExitStack
    from collections import deque

    xT_d = nc.dram_tensor("xT", [DIM, N], BF16, kind="ExternalInput")
    # wqk pre-shuffled on host: [pair, p, kc, 128] so each pair slice is a
    # single full-speed DMA (2KB contiguous rows)
    wq_d = nc.dram_tensor("wq", [4, P, KC, P], BF16, kind="ExternalInput")
    wk_d = nc.dram_tensor("wk", [4, P, KC, P], BF16, kind="ExternalInput")
    wv_d = nc.dram_tensor("wv", [4, P, KC, P], BF16, kind="ExternalInput")
    wo_d = nc.dram_tensor("wo", [FEAT, DIM], BF16, kind="ExternalInput")
    out_d = nc.dram_tensor("partial", [N, DIM], BF16, kind="ExternalOutput")

    with ExitStack() as ctx:
        big = ctx.enter_context(tc.tile_pool(name="big", bufs=1))

        # ---- persistent SBUF ----
        xT = big.tile([P, KC, N], BF16)  # 32KB/p
        v_aug = big.tile([P, NT, H_LOC, D + 1], BF16)  # 16.25KB/p
        OT = big.tile([P, 4, N], BF16)  # 16KB/p
        wv_sb = big.tile([P, 4, KC, P], BF16)  # 8KB/p
        wo_sb = big.tile([P, 4, DIM], BF16)  # 8KB/p
        wq_sb = big.tile([P, 4, KC, P], BF16)  # 8KB/p
        wk_sb = big.tile([P, 4, KC, P], BF16)  # 8KB/p

        # ones column of v_aug; zeroed warmup operand tile
        nc.vector.memset(v_aug[:, :, :, D], 1.0)
        warm = big.tile([P, 640], BF16)
        nc.vector.memset(warm[:], 0.0)
        ident = big.tile([P, P], BF16)
        from concourse import masks
        masks.make_identity(nc, ident[:])

        # ---- input DMAs (SP queue, in priority order) ----
        # xT in token-halves: the P0 projection (k/q tokens 0-1023) only
        # needs the first half of every kc chunk, so attention starts
        # ~6us after the 8 half-chunks land.
        nc.sync.dma_start(wq_sb[:, 0], wq_d.ap()[0])
        nc.sync.dma_start(wk_sb[:, 0], wk_d.ap()[0])
        for kc in range(KC):
            nc.sync.dma_start(
                xT[:, kc, 0:1024], xT_d.ap()[kc * P : (kc + 1) * P, 0:1024]
            )
        nc.sync.dma_start(wv_sb[:, 0], wv_d.ap()[0])
        for kc in range(KC):
            nc.sync.dma_start(
                xT[:, kc, 1024:N], xT_d.ap()[kc * P : (kc + 1) * P, 1024:N]
            )
        nc.sync.dma_start(wv_sb[:, 1], wv_d.ap()[1])
        nc.sync.dma_start(wq_sb[:, 1], wq_d.ap()[1])
        nc.sync.dma_start(wk_sb[:, 1], wk_d.ap()[1])
        nc.sync.dma_start(wv_sb[:, 2], wv_d.ap()[2])
        nc.sync.dma_start(wv_sb[:, 3], wv_d.ap()[3])
        nc.sync.dma_start(
            wo_sb[:], wo_d.ap().rearrange("(fc p) o -> p fc o", p=P)
        )
        for pr in (2, 3):
            nc.sync.dma_start(wq_sb[:, pr], wq_d.ap()[pr])
            nc.sync.dma_start(wk_sb[:, pr], wk_d.ap()[pr])

        # rotating pools
        qkT = ctx.enter_context(tc.tile_pool(name="qkT", bufs=2))
        exp_pool = ctx.enter_context(tc.tile_pool(name="exp", bufs=30))
        osb = ctx.enter_context(tc.tile_pool(name="osb", bufs=2))
        small = ctx.enter_context(tc.tile_pool(name="small", bufs=2))
        stage = ctx.enter_context(tc.tile_pool(name="stage", bufs=6))

        # ---- P0: the critical quarter of the pair-0 q/k projection ----
        # Only the dsts the first ~8 attention slots need: k tokens 0-1023
        # (j-chunks 0-7) and q tokens 0-1023 (the ib0 i-range).  kc-outer so
        # each xT chunk is consumed as it lands; the other 4 dsts drip into
        # the attention loop.  Warmup matmuls (zeroed operands, discarded
        # results in unused accumulator slots) keep the tensor engine's
        # p-state ramp alive across the DMA-paced stretches.
        DSTS0 = [("k", 0), ("q", 0), ("q", 1), ("k", 1)]
        DSTS1 = [("k", 2), ("q", 2), ("k", 3), ("q", 3)]

        def new_pair_tiles():
            qT = qkT.tile([P, N], BF16, tag="qT")
            kT = qkT.tile([P, N], BF16, tag="kT")
            return qT, kT

        pair_tiles = [None] * 4
        pair_tiles[0] = new_pair_tiles()
        with tc.tile_pool(name="p0ps", bufs=1, space="PSUM") as p0ps:
            acc = p0ps.tile([P, KC, 512], F32)

            def warmup(n):
                for wi in range(n):
                    nc.tensor.matmul(
                        acc[:, 4 + wi % 4], warm[:, 0:P], warm[:, 0:512],
                        start=True, stop=True,
                    )

            warmup(6)
            COPY = mybir.ActivationFunctionType.Copy
            for kc in range(KC):
                for di, (qk, blk) in enumerate(DSTS0):
                    w = wq_sb if qk == "q" else wk_sb
                    nc.tensor.matmul(
                        acc[:, di],
                        w[:, 0, kc],
                        xT[:, kc, blk * 512 : (blk + 1) * 512],
                        start=(kc == 0),
                        stop=(kc == KC - 1),
                    )
                    if kc == KC - 1:
                        dst = (
                            pair_tiles[0][0] if qk == "q" else pair_tiles[0][1]
                        )
                        dsl = dst[:, blk * 512 : (blk + 1) * 512]
                        if di < 2:
                            # ACT is idle pre-attention: route the two
                            # most critical copies there so the first ST
                            # isn't serialized behind one DVE queue
                            nc.scalar.activation(dsl, acc[:, di], COPY)
                        else:
                            nc.vector.tensor_copy(dsl, acc[:, di])
                if kc < 2:
                    warmup(1)

        # ---- P1 pools ----
        st_pool = ctx.enter_context(
            tc.tile_pool(name="st", bufs=2, space="PSUM")
        )
        oacc_pool = ctx.enter_context(
            tc.tile_pool(name="oacc", bufs=1, space="PSUM")
        )
        aux_pool = ctx.enter_context(
            tc.tile_pool(name="aux", bufs=2, space="PSUM")
        )

        # ---- drip generators (one PE matmul per yield) ----
        v_ready = [0, 0, 0, 0]  # per pair: number of token chunks projected

        def v_unit(pr, tc_i):
            # v for head pair pr (2 heads, 128 feat), token chunk tc_i
            ps = aux_pool.tile([P, P], F32, tag="aux", name=f"vps{pr}_{tc_i}")
            for kc in range(KC):
                nc.tensor.matmul(
                    ps[:],
                    xT[:, kc, tc_i * P : (tc_i + 1) * P],
                    wv_sb[:, pr, kc],
                    start=(kc == 0),
                    stop=(kc == KC - 1),
                )
                yield 53
            nc.vector.tensor_copy(
                v_aug[:, tc_i, 2 * pr : 2 * pr + 2, 0:D],
                ps[:].rearrange("p (h d) -> p h d", d=D),
            )
            v_ready[pr] = tc_i + 1

        proj_done = set()
        for qk, blk in DSTS0:
            proj_done.add((0, qk, blk))

        def proj_unit(pair, qk, blk):
            w = wq_sb if qk == "q" else wk_sb
            dst = pair_tiles[pair][0] if qk == "q" else pair_tiles[pair][1]
            ps = aux_pool.tile([P, 512], F32, tag="aux")
            for kc in range(KC):
                nc.tensor.matmul(
                    ps[:],
                    w[:, pair, kc],
                    xT[:, kc, blk * 512 : (blk + 1) * 512],
                    start=(kc == 0),
                    stop=(kc == KC - 1),
                )
                yield 213
            nc.vector.tensor_copy(dst[:, blk * 512 : (blk + 1) * 512], ps[:])
            proj_done.add((pair, qk, blk))

        def ensure_proj(pair, qk, blk):
            # correctness guard: an ST must never be emitted before the
            # projection unit writing its q/k slice
            while (pair, qk, blk) not in proj_done and fillers:
                pump_one()

        out_r = out_d.ap().rearrange("(tc p) o -> tc p o", p=P)

        def c_unit(tc_i, nb):
            ps = aux_pool.tile([P, 512], F32, tag="aux")
            for fc in range(4):
                nc.tensor.matmul(
                    ps[:],
                    OT[:, fc, tc_i * P : (tc_i + 1) * P],
                    wo_sb[:, fc, nb * 512 : (nb + 1) * 512],
                    start=(fc == 0),
                    stop=(fc == 3),
                )
                yield 213
            st = stage.tile([P, 512], BF16, tag="stg")
            nc.vector.tensor_copy(st[:], ps[:])
            nc.sync.dma_start(out_r[tc_i, :, nb * 512 : (nb + 1) * 512], st[:])

        fillers = deque()

        def pump_one():
            # returns the PE cost (ns) of the pumped step
            while fillers:
                try:
                    return next(fillers[0]) or 213
                except StopIteration:
                    fillers.popleft()
            return 0

        def drip(budget_ns):
            while budget_ns > 0 and fillers:
                c = pump_one()
                if c == 0:
                    break
                budget_ns -= c

        def ensure_v(pr, jc):
            # pump fillers until pair pr's v covers token chunk jc; the
            # fillers queue is ordered so this pulls a minimal prefix
            while v_ready[pr] <= jc and fillers:
                pump_one()

        def drain_all():
            while fillers:
                for _ in fillers.popleft():
                    pass

        # ---- attention block ----
        _bid = [0]

        def attn_block(h, i0, iw, o_pair, drip_rate, prev=None):
            """One head's attention for i in [i0, i0+iw).  iw in {512,1024}.

            Returns a `tail` list of closures (remaining PV emissions +
            the normalization) that the caller either flushes immediately
            or hands to the next block to drain one-per-slot.
            """
            pair, h2 = h // 2, h % 2
            prev = prev or []
            bid = _bid[0] = _bid[0] + 1
            qT, kT = pair_tiles[pair]
            qh = qT[h2 * D : (h2 + 1) * D]
            kh = kT[h2 * D : (h2 + 1) * D]
            nic = iw // P
            o_acc = oacc_pool.tile(
                [P, 8, D + 1], F32, tag="oacc", padded_shape=[P, 8, P]
            )
            exs = [None] * NT

            def pv_col(ic):
                # one full accumulation group per PSUM region: interleaving
                # several open groups within one PSUM bank corrupts the
                # accumulators, so each ic's 16 jc matmuls run as a unit
                ensure_v(pair, NT - 1)
                for jc in range(NT):
                    nc.tensor.matmul(
                        o_acc[:, ic, :],
                        exs[jc][:, ic * P : (ic + 1) * P],
                        v_aug[:, jc, h],
                        start=(jc == 0),
                        stop=(jc == NT - 1),
                    )

            def emit_norm():
                recip = small.tile([P, 8], F32, tag="recip")
                nc.vector.reciprocal(recip[:, 0:nic], o_acc[:, 0:nic, D])
                s0 = (i0 % 1024) // P
                nc.vector.tensor_mul(
                    o_pair[:, s0 : s0 + nic, h2, :],
                    o_acc[:, 0:nic, 0:D],
                    recip[:, 0:nic]
                    .rearrange("p (a b) -> p a b", b=1)
                    .to_broadcast([P, nic, D]),
                )

            for jc in range(NT):
                ensure_proj(pair, "k", jc // 4)
                for hf0 in range(iw // 512):
                    ensure_proj(pair, "q", i0 // 512 + hf0)
                st = st_pool.tile([P, iw], F32, tag="st")
                for hf in range(iw // 512):
                    nc.tensor.matmul(
                        st[:, hf * 512 : (hf + 1) * 512],
                        kh[:, jc * P : (jc + 1) * P],
                        qh[:, i0 + hf * 512 : i0 + (hf + 1) * 512],
                        start=True,
                        stop=True,
                    )
                ex = exp_pool.tile([P, iw], BF16, tag="ex")
                nc.scalar.activation(ex[:], st[:], EXP, scale=SCALE)
                exs[jc] = ex
                # drain earlier blocks' tails (PV columns + norms) at one
                # item per slot when backed up; drip only the remainder
                if prev and (len(prev) >= 5 or jc % 2 == 0):
                    prev.pop(0)[1]()
                    drip(60)
                else:
                    drip(drip_rate(jc))
            tail = [(bid, lambda ic=ic: pv_col(ic)) for ic in range(nic)]
            tail.append((bid, emit_norm))
            return (prev or []) + tail

        def transposes(pair, ib, o_pair, ic_range, on_pe=False):
            for ic in ic_range:
                dst = OT[:, pair, ib * 1024 + ic * P : ib * 1024 + (ic + 1) * P]
                src = o_pair[:, ic].rearrange("p a b -> p (a b)")
                if on_pe:
                    # low-latency path for the final blocks: PE transpose +
                    # DVE copy instead of the ~3.5us XBAR DMA chain
                    tp = aux_pool.tile([P, P], BF16, tag="aux", name=f"tp{ic}")
                    nc.tensor.transpose(tp[:], src, ident[:])
                    nc.vector.tensor_copy(dst, tp[:])
                else:
                    nc.sync.dma_start_transpose(dst, src)

        # ---- main loop ----
        # drip rates: heavy during h0 (v-projection), medium h1 (pair-1
        # proj), light elsewhere
        def mk_rate(base_ns, first_extra_ns=0):
            def rate(jc):
                return base_ns + (first_extra_ns if jc < 2 else 0)
            return rate

        carry = [[], None, 0]  # [tail, deferred ib1-transposes, their bid]
        for pair in range(4):
            if pair == 0:
                # pair-0 v interleaved with the deferred quarter of the
                # pair-0 q/k projection, ordered so ensure_v pulls a
                # minimal prefix while the deferred dsts land before the
                # ST j/i ranges that need them (k2 by jc8, k3 by jc12,
                # q2/q3 by ib1)
                defer = {2: proj_unit(0, "k", 2), 5: proj_unit(0, "k", 3),
                         7: proj_unit(0, "q", 2), 9: proj_unit(0, "q", 3)}
                for tc_i in range(NT):
                    fillers.append(v_unit(0, tc_i))
                    if tc_i in defer:
                        fillers.append(defer[tc_i])
            if pair < 3:
                pair_tiles[pair + 1] = new_pair_tiles()
                for qk, blk in (("q", 0), ("k", 0), ("q", 1), ("k", 1),
                                ("q", 2), ("k", 2), ("q", 3), ("k", 3)):
                    fillers.append(proj_unit(pair + 1, qk, blk))
                for tc_i in range(NT):
                    fillers.append(v_unit(pair + 1, tc_i))
            h_lo, h_hi = 2 * pair, 2 * pair + 1
            o_pairs = [
                osb.tile([P, 8, 2, D], BF16, tag="opair", name=f"op{pair}_{i}")
                for i in range(2)
            ]
            if pair == 0:
                rate = mk_rate(640, 300)
            elif pair == 3:
                rate = mk_rate(450, 350)
            else:
                rate = mk_rate(340, 350)
            last_pair = pair == 3

            def flush(tail, upto=None):
                while tail and (upto is None or tail[0][0] <= upto):
                    tail.pop(0)[1]()
                return tail

            if not last_pair:
                t = attn_block(h_lo, 0, 1024, o_pairs[0], rate, prev=carry[0])
                if carry[1] is not None:
                    t = flush(t, upto=carry[2])
                    carry[1]()
                t = attn_block(h_hi, 0, 1024, o_pairs[0], rate, prev=t)
                bid_b = _bid[0]
                t = attn_block(h_lo, 1024, 1024, o_pairs[1], rate, prev=t)
                t = flush(t, upto=bid_b)
                transposes(pair, 0, o_pairs[0], range(8))
                t = attn_block(h_hi, 1024, 1024, o_pairs[1], rate, prev=t)
                carry = [
                    t,
                    (lambda pr=pair, op=o_pairs[1]:
                        transposes(pr, 1, op, range(8))),
                    _bid[0],
                ]
            else:
                t = attn_block(h_lo, 0, 1024, o_pairs[0], rate, prev=carry[0])
                if carry[1] is not None:
                    t = flush(t, upto=carry[2])
                    carry[1]()
                t = attn_block(h_hi, 0, 1024, o_pairs[0], rate, prev=t)
                flush(t)
                transposes(pair, 0, o_pairs[0], range(8))
                # tokens 0..1023 complete across all pairs
                for tc_i in range(8):
                    for nb in range(2):
                        fillers.append(c_unit(tc_i, nb))
                # split the final head's ib1 to shrink the tail
                t = attn_block(h_lo, 1024, 1024, o_pairs[1], rate)
                t = attn_block(h_hi, 1024, 512, o_pairs[1], rate, prev=t)
                flush(t)
                transposes(pair, 1, o_pairs[1], range(4), on_pe=True)
                for tc_i in range(8, 12):
                    for nb in range(2):
                        fillers.append(c_unit(tc_i, nb))
                t = attn_block(h_hi, 1536, 512, o_pairs[1], rate)
                flush(t)
                transposes(pair, 1, o_pairs[1], range(4, 8), on_pe=True)
                for tc_i in range(12, 16):
                    for nb in range(2):
                        fillers.append(c_unit(tc_i, nb))
        drain_all()


def _build():
    nc = bacc.Bacc("TRN2", target_bir_lowering=False, debug=False)
    with nc.allow_low_precision(reason="bf16 attention within tolerance"):
        with tile.TileContext(nc) as tc:
            _emit(nc, tc)
    nc.compile()
    return nc


def _get_nc():
    if "nc" not in _CACHE:
        _CACHE["nc"] = _build()
    return _CACHE["nc"]


def kernel(x, w_qkv, w_out, b_out, _trace=False, _tmpdir=None):
    x = np.asarray(x, dtype=np.float32)
    w_qkv = np.asarray(w_qkv, dtype=np.float32)
    w_out = np.asarray(w_out, dtype=np.float32)
    b_out = np.asarray(b_out, dtype=np.float32)

    nc = _get_nc()
    in_maps = []
    for j in range(8):
        b, hg = j // 2, j % 2
        s = FEAT * hg
        wq = w_qkv[:, s : s + FEAT]
        wk = w_qkv[:, DIM + s : DIM + s + FEAT]
        wv = w_qkv[:, 2 * DIM + s : 2 * DIM + s + FEAT]
        # pre-shuffle wq/wk into [pair, p, kc, 128]
        shuf = lambda w: np.ascontiguousarray(
            w.reshape(KC, P, 4, P).transpose(2, 1, 0, 3)
        ).astype(bfloat16)
        in_maps.append(
            {
                "xT": np.ascontiguousarray(x[b].T).astype(bfloat16),
                "wq": shuf(wq),
                "wk": shuf(wk),
                "wv": shuf(wv),
                "wo": np.ascontiguousarray(w_out[s : s + FEAT, :]).astype(
                    bfloat16
                ),
            }
        )
    res = run_bass_kernel_spmd(
        nc, in_maps, core_ids=list(range(8)), trace=_trace, tmpdir=_tmpdir
    )
    out = np.empty((B, N, DIM), np.float32)
    for b in range(B):
        out[b] = res.results[2 * b]["partial"].astype(np.float32)
        out[b] += res.results[2 * b + 1]["partial"].astype(np.float32)
    out += b_out[None, None, :]
    if _trace:
        return out, res
    return out


# revision 3
# speedup vs baseline: 1.0007x; 1.0007x over previous
"""TRN2 Bass kernel for nn_Attention_56281251447235.

Multi-head attention: x:[4,2048,1024], w_qkv:[1024,3072] (q|k|v),
16 heads x 64 dim_head, w_out:[1024,1024], b_out:[1024].

Sharding over 8 NeuronCores: core j handles batch b=j//2 and head-group
hg=j%2 (8 of 16 heads).  Each core computes its 8 heads' attention and a
partial output projection [2048,1024] in bf16; the host sums the two
partials per batch in f32 and adds the bias.

All matmul operands bf16 (1 cycle/row at any free size).  Per-core
pipeline:
  P0  DMA wq0/wk0 + xT (kc-chunked) + remaining weights; pair-0 q/k
      projection kc-outer across an 8-bank PSUM pool so the last xT
      chunk is immediately followed by the last accumulation pass.
  P1  per pair p, per i-block ib (1024 wide; the very last head's ib1
      is split 2x512 to shrink the end-of-kernel dependency tail), per
      head h2:
        per jc (16 j-chunks of 128 tokens):
          S^T[j,i-block] = k_h j-chunk @ q_h  (PSUM, 2 matmuls of 512)
          ex = exp(S^T/8) on ACT -> SBUF bf16   (the pacing engine)
          PV: per 128-i chunk: o_acc[i,65] += ex_slice^T @ v_aug
              (v_aug has a ones column so col 64 accumulates the
              softmax denominator; M=i layout keeps the PE at
              128 outputs/cycle instead of 65/128 partitions)
        norm: DVE reciprocal of o_acc[:,:,64] + one broadcast multiply
              into O_pair[:,:,h2,:] bf16
      after both heads of (p, ib): 8 DMA-engine transposes
      [128i x (2x64)d] -> OT[:, p, i-slice]  (XBAR dma transpose)
  P2  output projection dripped into PE slack as OT columns become
      available; partial [2048,1024] bf16 DMA'd out per [128,512] tile.

Dripping: v-projection, pair p+1 q/k projection, and output-projection
units are generators yielding one PE matmul per step, pumped into the
attention loop's PE slack (ACT paces the inner loop at ~1038ns/jc vs
~644ns of PE work).
"""

import numpy as np
import ml_dtypes

import concourse.mybir as mybir
import concourse.tile as tile
from concourse import bacc
from concourse.bass_utils import run_bass_kernel_spmd

F32 = mybir.dt.float32
BF16 = mybir.dt.bfloat16
EXP = mybir.ActivationFunctionType.Exp
bfloat16 = ml_dtypes.bfloat16

P = 128
B, N, DIM = 4, 2048, 1024
H_LOC = 8  # heads per core
D = 64
FEAT = H_LOC * D  # 512
KC = DIM // P  # 8 contraction chunks
NT = N // P  # 16 token chunks (j-chunks)
SCALE = 1.0 / 8.0

_CACHE = {}


def _emit(nc, tc):
    from contextlib import ExitStack
    from collections import deque

    xT_d = nc.dram_tensor("xT", [DIM, N], BF16, kind="ExternalInput")
    # wqk pre-shuffled on host: [pair, p, kc, 128] so each pair slice is a
    # single full-speed DMA (2KB contiguous rows)
    wq_d = nc.dram_tensor("wq", [4, P, KC, P], BF16, kind="ExternalInput")
    wk_d = nc.dram_tensor("wk", [4, P, KC, P], BF16, kind="ExternalInput")
    wv_d = nc.dram_tensor("wv", [4, P, KC, P], BF16, kind="ExternalInput")
    wo_d = nc.dram_tensor("wo", [FEAT, DIM], BF16, kind="ExternalInput")
    out_d = nc.dram_tensor("partial", [N, DIM], BF16, kind="ExternalOutput")

    with ExitStack() as ctx:
        big = ctx.enter_context(tc.tile_pool(name="big", bufs=1))

        # ---- persistent SBUF ----
        xT = big.tile([P, KC, N], BF16)  # 32KB/p
        v_aug = big.tile([P, NT, H_LOC, D + 1], BF16)  # 16.25KB/p
        OT = big.tile([P, 4, N], BF16)  # 16KB/p
        wv_sb = big.tile([P, 4, KC, P], BF16)  # 8KB/p
        wo_sb = big.tile([P, 4, DIM], BF16)  # 8KB/p
        wq_sb = big.tile([P, 4, KC, P], BF16)  # 8KB/p
        wk_sb = big.tile([P, 4, KC, P], BF16)  # 8KB/p

        # ones column of v_aug; zeroed warmup operand tile
        nc.vector.memset(v_aug[:, :, :, D], 1.0)
        warm = big.tile([P, 640], BF16)
        nc.vector.memset(warm[:], 0.0)
        ident = big.tile([P, P], BF16)
        from concourse import masks
        masks.make_identity(nc, ident[:])

        # ---- input DMAs (SP queue, in priority order) ----
        # xT in token-halves: the P0 projection (k/q tokens 0-1023) only
        # needs the first half of every kc chunk, so attention starts
        # ~6us after the 8 half-chunks land.
        nc.sync.dma_start(wq_sb[:, 0], wq_d.ap()[0])
        nc.sync.dma_start(wk_sb[:, 0], wk_d.ap()[0])
        for kc in range(KC):
            nc.sync.dma_start(
                xT[:, kc, 0:1024], xT_d.ap()[kc * P : (kc + 1) * P, 0:1024]
            )
        nc.sync.dma_start(wv_sb[:, 0], wv_d.ap()[0])
        for kc in range(KC):
            nc.sync.dma_start(
                xT[:, kc, 1024:N], xT_d.ap()[kc * P : (kc + 1) * P, 1024:N]
            )
        nc.sync.dma_start(wv_sb[:, 1], wv_d.ap()[1])
        nc.sync.dma_start(wq_sb[:, 1], wq_d.ap()[1])
        nc.sync.dma_start(wk_sb[:, 1], wk_d.ap()[1])
        nc.sync.dma_start(wv_sb[:, 2], wv_d.ap()[2])
        nc.sync.dma_start(wv_sb[:, 3], wv_d.ap()[3])
        nc.sync.dma_start(
            wo_sb[:], wo_d.ap().rearrange("(fc p) o -> p fc o", p=P)
        )
        for pr in (2, 3):
            nc.sync.dma_start(wq_sb[:, pr], wq_d.ap()[pr])
            nc.sync.dma_start(wk_sb[:, pr], wk_d.ap()[pr])

        # rotating pools
        qkT = ctx.enter_context(tc.tile_pool(name="qkT", bufs=2))
        exp_pool = ctx.enter_context(tc.tile_pool(name="exp", bufs=30))
        osb = ctx.enter_context(tc.tile_pool(name="osb", bufs=2))
        small = ctx.enter_context(tc.tile_pool(name="small", bufs=2))
        stage = ctx.enter_context(tc.tile_pool(name="stage", bufs=6))

        # ---- P0: the critical quarter of the pair-0 q/k projection ----
        # Only the dsts the first ~8 attention slots need: k tokens 0-1023
        # (j-chunks 0-7) and q tokens 0-1023 (the ib0 i-range).  kc-outer so
        # each xT chunk is consumed as it lands; the other 4 dsts drip into
        # the attention loop.  Warmup matmuls (zeroed operands, discarded
        # results in unused accumulator slots) keep the tensor engine's
        # p-state ramp alive across the DMA-paced stretches.
        DSTS0 = [("k", 0), ("q", 0), ("q", 1), ("k", 1)]
        DSTS1 = [("k", 2), ("q", 2), ("k", 3), ("q", 3)]

        def new_pair_tiles():
            qT = qkT.tile([P, N], BF16, tag="qT")
            kT = qkT.tile([P, N], BF16, tag="kT")
            return qT, kT

        pair_tiles = [None] * 4
        pair_tiles[0] = new_pair_tiles()
        with tc.tile_pool(name="p0ps", bufs=1, space="PSUM") as p0ps:
            acc = p0ps.tile([P, KC, 512], F32)

            def warmup(n):
                for wi in range(n):
                    nc.tensor.matmul(
                        acc[:, 4 + wi % 4], warm[:, 0:P], warm[:, 0:512],
                        start=True, stop=True,
                    )

            warmup(6)
            COPY = mybir.ActivationFunctionType.Copy
            for kc in range(KC):
                for di, (qk, blk) in enumerate(DSTS0):
                    w = wq_sb if qk == "q" else wk_sb
                    nc.tensor.matmul(
                        acc[:, di],
                        w[:, 0, kc],
                        xT[:, kc, blk * 512 : (blk + 1) * 512],
                        start=(kc == 0),
                        stop=(kc == KC - 1),
                    )
                    if kc == KC - 1:
                        dst = (
                            pair_tiles[0][0] if qk == "q" else pair_tiles[0][1]
                        )
                        dsl = dst[:, blk * 512 : (blk + 1) * 512]
                        if di < 2:
                            # ACT is idle pre-attention: route the two
                            # most critical copies there so the first ST
                            # isn't serialized behind one DVE queue
                            nc.scalar.activation(dsl, acc[:, di], COPY)
                        else:
                            nc.vector.tensor_copy(dsl, acc[:, di])
                if kc < 2:
                    warmup(1)

        # ---- P1 pools ----
        st_pool = ctx.enter_context(
            tc.tile_pool(name="st", bufs=2, space="PSUM")
        )
        oacc_pool = ctx.enter_context(
            tc.tile_pool(name="oacc", bufs=1, space="PSUM")
        )
        aux_pool = ctx.enter_context(
            tc.tile_pool(name="aux", bufs=2, space="PSUM")
        )

        # ---- drip generators (one PE matmul per yield) ----
        v_ready = [0, 0, 0, 0]  # per pair: number of token chunks projected

        def v_unit(pr, tc_i):
            # v for head pair pr (2 heads, 128 feat), token chunk tc_i
            ps = aux_pool.tile([P, P], F32, tag="aux", name=f"vps{pr}_{tc_i}")
            for kc in range(KC):
                nc.tensor.matmul(
                    ps[:],
                    xT[:, kc, tc_i * P : (tc_i + 1) * P],
                    wv_sb[:, pr, kc],
                    start=(kc == 0),
                    stop=(kc == KC - 1),
                )
                yield 53
            nc.vector.tensor_copy(
                v_aug[:, tc_i, 2 * pr : 2 * pr + 2, 0:D],
                ps[:].rearrange("p (h d) -> p h d", d=D),
            )
            v_ready[pr] = tc_i + 1

        proj_done = set()
        for qk, blk in DSTS0:
            proj_done.add((0, qk, blk))

        def proj_unit(pair, qk, blk):
            w = wq_sb if qk == "q" else wk_sb
            dst = pair_tiles[pair][0] if qk == "q" else pair_tiles[pair][1]
            ps = aux_pool.tile([P, 512], F32, tag="aux")
            for kc in range(KC):
                nc.tensor.matmul(
                    ps[:],
                    w[:, pair, kc],
                    xT[:, kc, blk * 512 : (blk + 1) * 512],
                    start=(kc == 0),
                    stop=(kc == KC - 1),
                )
                yield 213
            nc.vector.tensor_copy(dst[:, blk * 512 : (blk + 1) * 512], ps[:])
            proj_done.add((pair, qk, blk))

        def ensure_proj(pair, qk, blk):
            # correctness guard: an ST must never be emitted before the
            # projection unit writing its q/k slice
            while (pair, qk, blk) not in proj_done and fillers:
                pump_one()

        out_r = out_d.ap().rearrange("(tc p) o -> tc p o", p=P)

        def c_unit(tc_i, nb):
            ps = aux_pool.tile([P, 512], F32, tag="aux")
            for fc in range(4):
                nc.tensor.matmul(
                    ps[:],
                    OT[:, fc, tc_i * P : (tc_i + 1) * P],
                    wo_sb[:, fc, nb * 512 : (nb + 1) * 512],
                    start=(fc == 0),
                    stop=(fc == 3),
                )
                yield 213
            st = stage.tile([P, 512], BF16, tag="stg")
            nc.vector.tensor_copy(st[:], ps[:])
            nc.sync.dma_start(out_r[tc_i, :, nb * 512 : (nb + 1) * 512], st[:])

        fillers = deque()

        def pump_one():
            # returns the PE cost (ns) of the pumped step
            while fillers:
                try:
                    return next(fillers[0]) or 213
                except StopIteration:
                    fillers.popleft()
            return 0

        def drip(budget_ns):
            while budget_ns > 0 and fillers:
                c = pump_one()
                if c == 0:
                    break
                budget_ns -= c

        def ensure_v(pr, jc):
            # pump fillers until pair pr's v covers token chunk jc; the
            # fillers queue is ordered so this pulls a minimal prefix
            while v_ready[pr] <= jc and fillers:
                pump_one()

        def drain_all():
            while fillers:
                for _ in fillers.popleft():
                    pass

        # ---- attention block ----
        _bid = [0]

        def attn_block(h, i0, iw, o_pair, drip_rate, prev=None, early=True):
            """One head's attention for i in [i0, i0+iw).  iw in {512,1024}.

            Returns a `tail` list of closures (remaining PV emissions +
            the normalization) that the caller either flushes immediately
            or hands to the next block to drain one-per-slot.
            """
            pair, h2 = h // 2, h % 2
            prev = prev or []
            bid = _bid[0] = _bid[0] + 1
            qT, kT = pair_tiles[pair]
            qh = qT[h2 * D : (h2 + 1) * D]
            kh = kT[h2 * D : (h2 + 1) * D]
            nic = iw // P
            o_acc = oacc_pool.tile(
                [P, 8, D + 1], F32, tag="oacc", padded_shape=[P, 8, P]
            )
            exs = [None] * NT

            def pv_half(ic, lo, hi):
                # accumulation groups must never interleave within one PSUM
                # bank: a column's group opens at jc=0 and closes at jc=15,
                # and columns are emitted strictly one after another (the
                # second half of column k always precedes the first half of
                # column k+1)
                ensure_v(pair, hi - 1)
                for jc in range(lo, hi):
                    nc.tensor.matmul(
                        o_acc[:, ic, :],
                        exs[jc][:, ic * P : (ic + 1) * P],
                        v_aug[:, jc, h],
                        start=(jc == 0),
                        stop=(jc == NT - 1),
                    )

            def pv_col(ic):
                pv_half(ic, 0, NT)

            def emit_norm():
                recip = small.tile([P, 8], F32, tag="recip")
                nc.vector.reciprocal(recip[:, 0:nic], o_acc[:, 0:nic, D])
                s0 = (i0 % 1024) // P
                nc.vector.tensor_mul(
                    o_pair[:, s0 : s0 + nic, h2, :],
                    o_acc[:, 0:nic, 0:D],
                    recip[:, 0:nic]
                    .rearrange("p (a b) -> p a b", b=1)
                    .to_broadcast([P, nic, D]),
                )

            col_cost = 16 * 65 * 0.4167  # ns per PV column
            for jc in range(NT):
                ensure_proj(pair, "k", jc // 4)
                for hf0 in range(iw // 512):
                    ensure_proj(pair, "q", i0 // 512 + hf0)
                st = st_pool.tile([P, iw], F32, tag="st")
                for hf in range(iw // 512):
                    nc.tensor.matmul(
                        st[:, hf * 512 : (hf + 1) * 512],
                        kh[:, jc * P : (jc + 1) * P],
                        qh[:, i0 + hf * 512 : i0 + (hf + 1) * 512],
                        start=True,
                        stop=True,
                    )
                ex = exp_pool.tile([P, iw], BF16, tag="ex")
                nc.scalar.activation(ex[:], st[:], EXP, scale=SCALE)
                exs[jc] = ex
                # drain earlier blocks' tails with cost-aware drip top-up so
                # zero-PE items (norms, transpose triggers) don't starve PE
                budget = drip_rate(jc)
                if prev and (len(prev) >= 5 or jc % 2 == 0):
                    item = prev.pop(0)
                    item[1]()
                    budget -= item[2]
                drip(max(60, budget))
            tail = [
                (bid, (lambda ic=ic: pv_col(ic)), col_cost)
                for ic in range(nic)
            ]
            tail.append((bid, emit_norm, 0))
            return (prev or []) + tail

        def transposes(pair, ib, o_pair, ic_range, on_pe=False):
            for ic in ic_range:
                dst = OT[:, pair, ib * 1024 + ic * P : ib * 1024 + (ic + 1) * P]
                src = o_pair[:, ic].rearrange("p a b -> p (a b)")
                if on_pe:
                    # low-latency path for the final blocks: PE transpose +
                    # DVE copy instead of the ~3.5us XBAR DMA chain
                    tp = aux_pool.tile([P, P], BF16, tag="aux", name=f"tp{ic}")
                    nc.tensor.transpose(tp[:], src, ident[:])
                    nc.vector.tensor_copy(dst, tp[:])
                else:
                    nc.sync.dma_start_transpose(dst, src)

        # ---- main loop ----
        # drip rates: heavy during h0 (v-projection), medium h1 (pair-1
        # proj), light elsewhere
        def mk_rate(base_ns, first_extra_ns=0):
            def rate(jc):
                return base_ns + (first_extra_ns if jc < 2 else 0)
            return rate

        carry = [[], None, 0]  # [tail, deferred ib1-transposes, their bid]
        for pair in range(4):
            if pair == 0:
                # pair-0 v interleaved with the deferred quarter of the
                # pair-0 q/k projection, ordered so ensure_v pulls a
                # minimal prefix while the deferred dsts land before the
                # ST j/i ranges that need them (k2 by jc8, k3 by jc12,
                # q2/q3 by ib1)
                defer = {2: proj_unit(0, "k", 2), 5: proj_unit(0, "k", 3),
                         7: proj_unit(0, "q", 2), 9: proj_unit(0, "q", 3)}
                for tc_i in range(NT):
                    fillers.append(v_unit(0, tc_i))
                    if tc_i in defer:
                        fillers.append(defer[tc_i])
            if pair < 3:
                pair_tiles[pair + 1] = new_pair_tiles()
                for qk, blk in (("q", 0), ("k", 0), ("q", 1), ("k", 1),
                                ("q", 2), ("k", 2), ("q", 3), ("k", 3)):
                    fillers.append(proj_unit(pair + 1, qk, blk))
                for tc_i in range(NT):
                    fillers.append(v_unit(pair + 1, tc_i))
            h_lo, h_hi = 2 * pair, 2 * pair + 1
            o_pairs = [
                osb.tile([P, 8, 2, D], BF16, tag="opair", name=f"op{pair}_{i}")
                for i in range(2)
            ]
            if pair == 0:
                rate = mk_rate(640, 300)
            elif pair == 3:
                rate = mk_rate(450, 350)
            else:
                rate = mk_rate(340, 350)
            last_pair = pair == 3

            def flush(tail, upto=None):
                while tail and (upto is None or tail[0][0] <= upto):
                    tail.pop(0)[1]()
                return tail

            if not last_pair:
                t = attn_block(h_lo, 0, 1024, o_pairs[0], rate, prev=carry[0],
                               early=(pair > 0))
                if carry[1] is not None:
                    t = flush(t, upto=carry[2])
                    carry[1]()
                t = attn_block(h_hi, 0, 1024, o_pairs[0], rate, prev=t)
                bid_b = _bid[0]
                t = attn_block(h_lo, 1024, 1024, o_pairs[1], rate, prev=t)
                t = flush(t, upto=bid_b)
                transposes(pair, 0, o_pairs[0], range(8))
                t = attn_block(h_hi, 1024, 1024, o_pairs[1], rate, prev=t)
                carry = [
                    t,
                    (lambda pr=pair, op=o_pairs[1]:
                        transposes(pr, 1, op, range(8))),
                    _bid[0],
                ]
            else:
                def tp_ib0():
                    transposes(pair, 0, o_pairs[0], range(8))
                    # tokens 0..1023 complete across all pairs
                    for tc_i in range(8):
                        for nb in range(2):
                            fillers.append(c_unit(tc_i, nb))

                def tp_ib1a():
                    transposes(pair, 1, o_pairs[1], range(4), on_pe=True)
                    for tc_i in range(8, 12):
                        for nb in range(2):
                            fillers.append(c_unit(tc_i, nb))

                t = attn_block(h_lo, 0, 1024, o_pairs[0], rate, prev=carry[0])
                if carry[1] is not None:
                    t = flush(t, upto=carry[2])
                    carry[1]()
                t = attn_block(h_hi, 0, 1024, o_pairs[0], rate, prev=t)
                t.append((_bid[0], tp_ib0, 0))
                # split the final head's ib1 to shrink the tail
                t = attn_block(h_lo, 1024, 1024, o_pairs[1], rate, prev=t)
                t = attn_block(h_hi, 1024, 512, o_pairs[1], rate, prev=t)
                t.append((_bid[0], tp_ib1a, 0))
                t = attn_block(h_hi, 1536, 512, o_pairs[1], rate, prev=t)
                flush(t)
                transposes(pair, 1, o_pairs[1], range(4, 8), on_pe=True)
                for tc_i in range(12, 16):
                    for nb in range(2):
                        fillers.append(c_unit(tc_i, nb))
        drain_all()


def _build():
    nc = bacc.Bacc("TRN2", target_bir_lowering=False, debug=False)
    with nc.allow_low_precision(reason="bf16 attention within tolerance"):
        with tile.TileContext(nc) as tc:
            _emit(nc, tc)
    nc.compile()
    return nc


def _get_nc():
    if "nc" not in _CACHE:
        _CACHE["nc"] = _build()
    return _CACHE["nc"]


def kernel(x, w_qkv, w_out, b_out, _trace=False, _tmpdir=None):
    x = np.asarray(x, dtype=np.float32)
    w_qkv = np.asarray(w_qkv, dtype=np.float32)
    w_out = np.asarray(w_out, dtype=np.float32)
    b_out = np.asarray(b_out, dtype=np.float32)

    nc = _get_nc()
    in_maps = []
    for j in range(8):
        b, hg = j // 2, j % 2
        s = FEAT * hg
        wq = w_qkv[:, s : s + FEAT]
        wk = w_qkv[:, DIM + s : DIM + s + FEAT]
        wv = w_qkv[:, 2 * DIM + s : 2 * DIM + s + FEAT]
        # pre-shuffle wq/wk into [pair, p, kc, 128]
        shuf = lambda w: np.ascontiguousarray(
            w.reshape(KC, P, 4, P).transpose(2, 1, 0, 3)
        ).astype(bfloat16)
        in_maps.append(
            {
                "xT": np.ascontiguousarray(x[b].T).astype(bfloat16),
                "wq": shuf(wq),
                "wk": shuf(wk),
                "wv": shuf(wv),
                "wo": np.ascontiguousarray(w_out[s : s + FEAT, :]).astype(
                    bfloat16
                ),
            }
        )
    res = run_bass_kernel_spmd(
        nc, in_maps, core_ids=list(range(8)), trace=_trace, tmpdir=_tmpdir
    )
    out = np.empty((B, N, DIM), np.float32)
    for b in range(B):
        out[b] = res.results[2 * b]["partial"].astype(np.float32)
        out[b] += res.results[2 * b + 1]["partial"].astype(np.float32)
    out += b_out[None, None, :]
    if _trace:
        return out, res
    return out


# revision 4
# speedup vs baseline: 1.0171x; 1.0164x over previous
"""TRN2 Bass kernel for nn_Attention_56281251447235.

Multi-head attention: x:[4,2048,1024], w_qkv:[1024,3072] (q|k|v),
16 heads x 64 dim_head, w_out:[1024,1024], b_out:[1024].

Sharding over 8 NeuronCores: core j handles batch b=j//2 and head-group
hg=j%2 (8 of 16 heads).  Each core computes its 8 heads' attention and a
partial output projection [2048,1024] in bf16; the host sums the two
partials per batch in f32 and adds the bias.

All matmul operands bf16 (1 cycle/row at any free size).  Per-core
pipeline:
  P0  DMA wq0/wk0 + xT (kc-chunked) + remaining weights; pair-0 q/k
      projection kc-outer across an 8-bank PSUM pool so the last xT
      chunk is immediately followed by the last accumulation pass.
  P1  per pair p, per i-block ib (1024 wide; the very last head's ib1
      is split 2x512 to shrink the end-of-kernel dependency tail), per
      head h2:
        per jc (16 j-chunks of 128 tokens):
          S^T[j,i-block] = k_h j-chunk @ q_h  (PSUM, 2 matmuls of 512)
          ex = exp(S^T/8) on ACT -> SBUF bf16   (the pacing engine)
          PV: per 128-i chunk: o_acc[i,65] += ex_slice^T @ v_aug
              (v_aug has a ones column so col 64 accumulates the
              softmax denominator; M=i layout keeps the PE at
              128 outputs/cycle instead of 65/128 partitions)
        norm: DVE reciprocal of o_acc[:,:,64] + one broadcast multiply
              into O_pair[:,:,h2,:] bf16
      after both heads of (p, ib): 8 DMA-engine transposes
      [128i x (2x64)d] -> OT[:, p, i-slice]  (XBAR dma transpose)
  P2  output projection dripped into PE slack as OT columns become
      available; partial [2048,1024] bf16 DMA'd out per [128,512] tile.

Dripping: v-projection, pair p+1 q/k projection, and output-projection
units are generators yielding one PE matmul per step, pumped into the
attention loop's PE slack (ACT paces the inner loop at ~1038ns/jc vs
~644ns of PE work).
"""

import numpy as np
import ml_dtypes

import concourse.mybir as mybir
import concourse.tile as tile
from concourse import bacc
from concourse.bass_utils import run_bass_kernel_spmd

F32 = mybir.dt.float32
BF16 = mybir.dt.bfloat16
EXP = mybir.ActivationFunctionType.Exp
bfloat16 = ml_dtypes.bfloat16

P = 128
B, N, DIM = 4, 2048, 1024
H_LOC = 8  # heads per core
D = 64
FEAT = H_LOC * D  # 512
KC = DIM // P  # 8 contraction chunks
NT = N // P  # 16 token chunks (j-chunks)
SCALE = 1.0 / 8.0

_CACHE = {}


def _emit(nc, tc):
    from contextlib import ExitStack
    from collections import deque

    xT_d = nc.dram_tensor("xT", [DIM, N], BF16, kind="ExternalInput")
    # wqk pre-shuffled on host: [pair, p, kc, 128] so each pair slice is a
    # single full-speed DMA (2KB contiguous rows)
    wq_d = nc.dram_tensor("wq", [4, P, KC, P], BF16, kind="ExternalInput")
    wk_d = nc.dram_tensor("wk", [4, P, KC, P], BF16, kind="ExternalInput")
    wv_d = nc.dram_tensor("wv", [4, P, KC, P], BF16, kind="ExternalInput")
    wo_d = nc.dram_tensor("wo", [FEAT, DIM], BF16, kind="ExternalInput")
    out_d = nc.dram_tensor("partial", [N, DIM], BF16, kind="ExternalOutput")

    with ExitStack() as ctx:
        big = ctx.enter_context(tc.tile_pool(name="big", bufs=1))

        # ---- persistent SBUF ----
        xT = big.tile([P, KC, N], BF16)  # 32KB/p
        v_aug = big.tile([P, NT, H_LOC, D + 1], BF16)  # 16.25KB/p
        OT = big.tile([P, 4, N], BF16)  # 16KB/p
        wv_sb = big.tile([P, 4, KC, P], BF16)  # 8KB/p
        wo_sb = big.tile([P, 4, DIM], BF16)  # 8KB/p
        wq_sb = big.tile([P, 4, KC, P], BF16)  # 8KB/p
        wk_sb = big.tile([P, 4, KC, P], BF16)  # 8KB/p

        # ones column of v_aug; zeroed warmup operand tile
        nc.vector.memset(v_aug[:, :, :, D], 1.0)
        warm = big.tile([P, 640], BF16)
        nc.vector.memset(warm[:], 0.0)
        ident = big.tile([P, P], BF16)
        from concourse import masks
        masks.make_identity(nc, ident[:])

        # ---- input DMAs (SP queue, in priority order) ----
        # xT in token-halves: the P0 projection (k/q tokens 0-1023) only
        # needs the first half of every kc chunk, so attention starts
        # ~6us after the 8 half-chunks land.
        nc.sync.dma_start(wq_sb[:, 0], wq_d.ap()[0])
        nc.sync.dma_start(wk_sb[:, 0], wk_d.ap()[0])
        for kc in range(KC):
            nc.sync.dma_start(
                xT[:, kc, 0:1024], xT_d.ap()[kc * P : (kc + 1) * P, 0:1024]
            )
        nc.sync.dma_start(wv_sb[:, 0], wv_d.ap()[0])
        for kc in range(KC):
            nc.sync.dma_start(
                xT[:, kc, 1024:N], xT_d.ap()[kc * P : (kc + 1) * P, 1024:N]
            )
        nc.sync.dma_start(wv_sb[:, 1], wv_d.ap()[1])
        nc.sync.dma_start(wq_sb[:, 1], wq_d.ap()[1])
        nc.sync.dma_start(wk_sb[:, 1], wk_d.ap()[1])
        nc.sync.dma_start(wv_sb[:, 2], wv_d.ap()[2])
        nc.sync.dma_start(wv_sb[:, 3], wv_d.ap()[3])
        nc.sync.dma_start(
            wo_sb[:], wo_d.ap().rearrange("(fc p) o -> p fc o", p=P)
        )
        for pr in (2, 3):
            nc.sync.dma_start(wq_sb[:, pr], wq_d.ap()[pr])
            nc.sync.dma_start(wk_sb[:, pr], wk_d.ap()[pr])

        # rotating pools
        qkT = ctx.enter_context(tc.tile_pool(name="qkT", bufs=2))
        exp_pool = ctx.enter_context(tc.tile_pool(name="exp", bufs=30))
        osb = ctx.enter_context(tc.tile_pool(name="osb", bufs=2))
        small = ctx.enter_context(tc.tile_pool(name="small", bufs=2))
        stage = ctx.enter_context(tc.tile_pool(name="stage", bufs=6))

        # ---- P0: the critical quarter of the pair-0 q/k projection ----
        # Only the dsts the first ~8 attention slots need: k tokens 0-1023
        # (j-chunks 0-7) and q tokens 0-1023 (the ib0 i-range).  kc-outer so
        # each xT chunk is consumed as it lands; the other 4 dsts drip into
        # the attention loop.  Warmup matmuls (zeroed operands, discarded
        # results in unused accumulator slots) keep the tensor engine's
        # p-state ramp alive across the DMA-paced stretches.
        DSTS0 = [("k", 0), ("q", 0), ("q", 1), ("k", 1)]
        DSTS1 = [("k", 2), ("q", 2), ("k", 3), ("q", 3)]

        def new_pair_tiles():
            qT = qkT.tile([P, N], BF16, tag="qT")
            kT = qkT.tile([P, N], BF16, tag="kT")
            return qT, kT

        pair_tiles = [None] * 4
        pair_tiles[0] = new_pair_tiles()
        with tc.tile_pool(name="p0ps", bufs=1, space="PSUM") as p0ps:
            acc = p0ps.tile([P, KC, 512], F32)

            def warmup(n):
                for wi in range(n):
                    nc.tensor.matmul(
                        acc[:, 4 + wi % 4], warm[:, 0:P], warm[:, 0:512],
                        start=True, stop=True,
                    )

            warmup(6)
            COPY = mybir.ActivationFunctionType.Copy
            for kc in range(KC):
                for di, (qk, blk) in enumerate(DSTS0):
                    w = wq_sb if qk == "q" else wk_sb
                    nc.tensor.matmul(
                        acc[:, di],
                        w[:, 0, kc],
                        xT[:, kc, blk * 512 : (blk + 1) * 512],
                        start=(kc == 0),
                        stop=(kc == KC - 1),
                    )
                    if kc == KC - 1:
                        dst = (
                            pair_tiles[0][0] if qk == "q" else pair_tiles[0][1]
                        )
                        dsl = dst[:, blk * 512 : (blk + 1) * 512]
                        if di % 2 == 0:
                            # ACT is idle pre-attention: alternate the
                            # copies between ACT and DVE so the first ST's
                            # operands (k0 on ACT, q0 on DVE) land in
                            # parallel rather than serialized on one queue
                            nc.scalar.activation(dsl, acc[:, di], COPY)
                        else:
                            nc.vector.tensor_copy(dsl, acc[:, di])
                if kc < 2:
                    warmup(1)

        # ---- P1 pools ----
        st_pool = ctx.enter_context(
            tc.tile_pool(name="st", bufs=2, space="PSUM")
        )
        oacc_pool = ctx.enter_context(
            tc.tile_pool(name="oacc", bufs=1, space="PSUM")
        )
        aux_pool = ctx.enter_context(
            tc.tile_pool(name="aux", bufs=2, space="PSUM")
        )

        # ---- drip generators (one PE matmul per yield) ----
        v_ready = [0, 0, 0, 0]  # per pair: number of token chunks projected

        def v_unit(pr, tc_i):
            # v for head pair pr (2 heads, 128 feat), token chunk tc_i
            ps = aux_pool.tile([P, P], F32, tag="aux", name=f"vps{pr}_{tc_i}")
            for kc in range(KC):
                nc.tensor.matmul(
                    ps[:],
                    xT[:, kc, tc_i * P : (tc_i + 1) * P],
                    wv_sb[:, pr, kc],
                    start=(kc == 0),
                    stop=(kc == KC - 1),
                )
                yield 53
            nc.vector.tensor_copy(
                v_aug[:, tc_i, 2 * pr : 2 * pr + 2, 0:D],
                ps[:].rearrange("p (h d) -> p h d", d=D),
            )
            v_ready[pr] = tc_i + 1

        proj_done = set()
        for qk, blk in DSTS0:
            proj_done.add((0, qk, blk))

        def proj_unit(pair, qk, blk):
            w = wq_sb if qk == "q" else wk_sb
            dst = pair_tiles[pair][0] if qk == "q" else pair_tiles[pair][1]
            ps = aux_pool.tile([P, 512], F32, tag="aux")
            for kc in range(KC):
                nc.tensor.matmul(
                    ps[:],
                    w[:, pair, kc],
                    xT[:, kc, blk * 512 : (blk + 1) * 512],
                    start=(kc == 0),
                    stop=(kc == KC - 1),
                )
                yield 213
            nc.vector.tensor_copy(dst[:, blk * 512 : (blk + 1) * 512], ps[:])
            proj_done.add((pair, qk, blk))

        def ensure_proj(pair, qk, blk):
            # correctness guard: an ST must never be emitted before the
            # projection unit writing its q/k slice
            while (pair, qk, blk) not in proj_done and fillers:
                pump_one()

        out_r = out_d.ap().rearrange("(tc p) o -> tc p o", p=P)

        def c_unit(tc_i, nb):
            ps = aux_pool.tile([P, 512], F32, tag="aux")
            for fc in range(4):
                nc.tensor.matmul(
                    ps[:],
                    OT[:, fc, tc_i * P : (tc_i + 1) * P],
                    wo_sb[:, fc, nb * 512 : (nb + 1) * 512],
                    start=(fc == 0),
                    stop=(fc == 3),
                )
                yield 213
            st = stage.tile([P, 512], BF16, tag="stg")
            nc.vector.tensor_copy(st[:], ps[:])
            nc.sync.dma_start(out_r[tc_i, :, nb * 512 : (nb + 1) * 512], st[:])

        fillers = deque()

        def pump_one():
            # returns the PE cost (ns) of the pumped step
            while fillers:
                try:
                    return next(fillers[0]) or 213
                except StopIteration:
                    fillers.popleft()
            return 0

        def drip(budget_ns):
            while budget_ns > 0 and fillers:
                c = pump_one()
                if c == 0:
                    break
                budget_ns -= c

        def ensure_v(pr, jc):
            # pump fillers until pair pr's v covers token chunk jc; the
            # fillers queue is ordered so this pulls a minimal prefix
            while v_ready[pr] <= jc and fillers:
                pump_one()

        def drain_all():
            while fillers:
                for _ in fillers.popleft():
                    pass

        # ---- attention block ----
        _bid = [0]

        def attn_block(h, i0, iw, o_pair, drip_rate, prev=None, early=True):
            """One head's attention for i in [i0, i0+iw).  iw in {512,1024}.

            Returns a `tail` list of closures (remaining PV emissions +
            the normalization) that the caller either flushes immediately
            or hands to the next block to drain one-per-slot.
            """
            pair, h2 = h // 2, h % 2
            prev = prev or []
            bid = _bid[0] = _bid[0] + 1
            qT, kT = pair_tiles[pair]
            qh = qT[h2 * D : (h2 + 1) * D]
            kh = kT[h2 * D : (h2 + 1) * D]
            nic = iw // P
            o_acc = oacc_pool.tile(
                [P, 8, D + 1], F32, tag="oacc", padded_shape=[P, 8, P]
            )
            exs = [None] * NT

            def pv_half(ic, lo, hi):
                # accumulation groups must never interleave within one PSUM
                # bank: a column's group opens at jc=0 and closes at jc=15,
                # and columns are emitted strictly one after another (the
                # second half of column k always precedes the first half of
                # column k+1)
                ensure_v(pair, hi - 1)
                for jc in range(lo, hi):
                    nc.tensor.matmul(
                        o_acc[:, ic, :],
                        exs[jc][:, ic * P : (ic + 1) * P],
                        v_aug[:, jc, h],
                        start=(jc == 0),
                        stop=(jc == NT - 1),
                    )

            def pv_col(ic):
                pv_half(ic, 0, NT)

            def emit_norm():
                recip = small.tile([P, 8], F32, tag="recip")
                nc.vector.reciprocal(recip[:, 0:nic], o_acc[:, 0:nic, D])
                s0 = (i0 % 1024) // P
                nc.vector.tensor_mul(
                    o_pair[:, s0 : s0 + nic, h2, :],
                    o_acc[:, 0:nic, 0:D],
                    recip[:, 0:nic]
                    .rearrange("p (a b) -> p a b", b=1)
                    .to_broadcast([P, nic, D]),
                )

            col_cost = 16 * 65 * 0.4167  # ns per PV column
            for jc in range(NT):
                ensure_proj(pair, "k", jc // 4)
                for hf0 in range(iw // 512):
                    ensure_proj(pair, "q", i0 // 512 + hf0)
                st = st_pool.tile([P, iw], F32, tag="st")
                for hf in range(iw // 512):
                    nc.tensor.matmul(
                        st[:, hf * 512 : (hf + 1) * 512],
                        kh[:, jc * P : (jc + 1) * P],
                        qh[:, i0 + hf * 512 : i0 + (hf + 1) * 512],
                        start=True,
                        stop=True,
                    )
                ex = exp_pool.tile([P, iw], BF16, tag="ex")
                nc.scalar.activation(ex[:], st[:], EXP, scale=SCALE)
                exs[jc] = ex
                # drain earlier blocks' tails with cost-aware drip top-up so
                # zero-PE items (norms, transpose triggers) don't starve PE
                budget = drip_rate(jc)
                if prev and (len(prev) >= 5 or jc % 2 == 0):
                    item = prev.pop(0)
                    item[1]()
                    budget -= item[2]
                drip(max(60, budget))
            tail = [
                (bid, (lambda ic=ic: pv_col(ic)), col_cost)
                for ic in range(nic)
            ]
            tail.append((bid, emit_norm, 0))
            return (prev or []) + tail

        def transposes(pair, ib, o_pair, ic_range, on_pe=False):
            for ic in ic_range:
                dst = OT[:, pair, ib * 1024 + ic * P : ib * 1024 + (ic + 1) * P]
                src = o_pair[:, ic].rearrange("p a b -> p (a b)")
                if on_pe:
                    # low-latency path for the final blocks: PE transpose +
                    # DVE copy instead of the ~3.5us XBAR DMA chain
                    tp = aux_pool.tile([P, P], BF16, tag="aux", name=f"tp{ic}")
                    nc.tensor.transpose(tp[:], src, ident[:])
                    nc.vector.tensor_copy(dst, tp[:])
                else:
                    nc.sync.dma_start_transpose(dst, src)

        # ---- main loop ----
        # drip rates: heavy during h0 (v-projection), medium h1 (pair-1
        # proj), light elsewhere
        def mk_rate(base_ns, first_extra_ns=0):
            def rate(jc):
                return base_ns + (first_extra_ns if jc < 2 else 0)
            return rate

        carry = [[], None, 0]  # [tail, deferred ib1-transposes, their bid]
        for pair in range(4):
            if pair == 0:
                # pair-0 v interleaved with the deferred quarter of the
                # pair-0 q/k projection, ordered so ensure_v pulls a
                # minimal prefix while the deferred dsts land before the
                # ST j/i ranges that need them (k2 by jc8, k3 by jc12,
                # q2/q3 by ib1)
                defer = {2: proj_unit(0, "k", 2), 5: proj_unit(0, "k", 3),
                         7: proj_unit(0, "q", 2), 9: proj_unit(0, "q", 3)}
                for tc_i in range(NT):
                    fillers.append(v_unit(0, tc_i))
                    if tc_i in defer:
                        fillers.append(defer[tc_i])
            if pair < 3:
                pair_tiles[pair + 1] = new_pair_tiles()
                for qk, blk in (("q", 0), ("k", 0), ("q", 1), ("k", 1),
                                ("q", 2), ("k", 2), ("q", 3), ("k", 3)):
                    fillers.append(proj_unit(pair + 1, qk, blk))
                for tc_i in range(NT):
                    fillers.append(v_unit(pair + 1, tc_i))
            h_lo, h_hi = 2 * pair, 2 * pair + 1
            o_pairs = [
                osb.tile([P, 8, 2, D], BF16, tag="opair", name=f"op{pair}_{i}")
                for i in range(2)
            ]
            if pair == 0:
                rate = mk_rate(640, 300)
            elif pair == 3:
                rate = mk_rate(450, 350)
            else:
                rate = mk_rate(340, 350)
            last_pair = pair == 3

            def flush(tail, upto=None):
                while tail and (upto is None or tail[0][0] <= upto):
                    tail.pop(0)[1]()
                return tail

            if not last_pair:
                t = attn_block(h_lo, 0, 1024, o_pairs[0], rate, prev=carry[0],
                               early=(pair > 0))
                if carry[1] is not None:
                    t = flush(t, upto=carry[2])
                    carry[1]()
                t = attn_block(h_hi, 0, 1024, o_pairs[0], rate, prev=t)
                bid_b = _bid[0]
                t = attn_block(h_lo, 1024, 1024, o_pairs[1], rate, prev=t)
                t = flush(t, upto=bid_b)
                transposes(pair, 0, o_pairs[0], range(8))
                t = attn_block(h_hi, 1024, 1024, o_pairs[1], rate, prev=t)
                carry = [
                    t,
                    (lambda pr=pair, op=o_pairs[1]:
                        transposes(pr, 1, op, range(8))),
                    _bid[0],
                ]
            else:
                def tp_ib0():
                    transposes(pair, 0, o_pairs[0], range(8))
                    # tokens 0..1023 complete across all pairs
                    for tc_i in range(8):
                        for nb in range(2):
                            fillers.append(c_unit(tc_i, nb))

                def tp_ib1a():
                    transposes(pair, 1, o_pairs[1], range(4), on_pe=True)
                    for tc_i in range(8, 12):
                        for nb in range(2):
                            fillers.append(c_unit(tc_i, nb))

                t = attn_block(h_lo, 0, 1024, o_pairs[0], rate, prev=carry[0])
                if carry[1] is not None:
                    t = flush(t, upto=carry[2])
                    carry[1]()
                t = attn_block(h_hi, 0, 1024, o_pairs[0], rate, prev=t)
                t.append((_bid[0], tp_ib0, 0))
                # split the final head's ib1 to shrink the tail
                t = attn_block(h_lo, 1024, 1024, o_pairs[1], rate, prev=t)
                t = attn_block(h_hi, 1024, 512, o_pairs[1], rate, prev=t)
                t.append((_bid[0], tp_ib1a, 0))
                t = attn_block(h_hi, 1536, 512, o_pairs[1], rate, prev=t)
                flush(t)
                transposes(pair, 1, o_pairs[1], range(4, 8), on_pe=True)
                for tc_i in range(12, 16):
                    for nb in range(2):
                        fillers.append(c_unit(tc_i, nb))
        drain_all()


def _build():
    nc = bacc.Bacc("TRN2", target_bir_lowering=False, debug=False)
    with nc.allow_low_precision(reason="bf16 attention within tolerance"):
        with tile.TileContext(nc) as tc:
            _emit(nc, tc)
    nc.compile()
    return nc


def _get_nc():
    if "nc" not in _CACHE:
        _CACHE["nc"] = _build()
    return _CACHE["nc"]


def kernel(x, w_qkv, w_out, b_out, _trace=False, _tmpdir=None):
    x = np.asarray(x, dtype=np.float32)
    w_qkv = np.asarray(w_qkv, dtype=np.float32)
    w_out = np.asarray(w_out, dtype=np.float32)
    b_out = np.asarray(b_out, dtype=np.float32)

    nc = _get_nc()
    in_maps = []
    for j in range(8):
        b, hg = j // 2, j % 2
        s = FEAT * hg
        wq = w_qkv[:, s : s + FEAT]
        wk = w_qkv[:, DIM + s : DIM + s + FEAT]
        wv = w_qkv[:, 2 * DIM + s : 2 * DIM + s + FEAT]
        # pre-shuffle wq/wk into [pair, p, kc, 128]
        shuf = lambda w: np.ascontiguousarray(
            w.reshape(KC, P, 4, P).transpose(2, 1, 0, 3)
        ).astype(bfloat16)
        in_maps.append(
            {
                "xT": np.ascontiguousarray(x[b].T).astype(bfloat16),
                "wq": shuf(wq),
                "wk": shuf(wk),
                "wv": shuf(wv),
                "wo": np.ascontiguousarray(w_out[s : s + FEAT, :]).astype(
                    bfloat16
                ),
            }
        )
    res = run_bass_kernel_spmd(
        nc, in_maps, core_ids=list(range(8)), trace=_trace, tmpdir=_tmpdir
    )
    out = np.empty((B, N, DIM), np.float32)
    for b in range(B):
        out[b] = res.results[2 * b]["partial"].astype(np.float32)
        out[b] += res.results[2 * b + 1]["partial"].astype(np.float32)
    out += b_out[None, None, :]
    if _trace:
        return out, res
    return out


# revision 5
# speedup vs baseline: 1.0176x; 1.0005x over previous
"""TRN2 Bass kernel for nn_Attention_56281251447235.

Multi-head attention: x:[4,2048,1024], w_qkv:[1024,3072] (q|k|v),
16 heads x 64 dim_head, w_out:[1024,1024], b_out:[1024].

Sharding over 8 NeuronCores: core j handles batch b=j//2 and head-group
hg=j%2 (8 of 16 heads).  Each core computes its 8 heads' attention and a
partial output projection [2048,1024] in bf16; the host sums the two
partials per batch in f32 and adds the bias.

All matmul operands bf16 (1 cycle/row at any free size).  Per-core
pipeline:
  P0  DMA wq0/wk0 + xT (kc-chunked) + remaining weights; pair-0 q/k
      projection kc-outer across an 8-bank PSUM pool so the last xT
      chunk is immediately followed by the last accumulation pass.
  P1  per pair p, per i-block ib (1024 wide; the very last head's ib1
      is split 2x512 to shrink the end-of-kernel dependency tail), per
      head h2:
        per jc (16 j-chunks of 128 tokens):
          S^T[j,i-block] = k_h j-chunk @ q_h  (PSUM, 2 matmuls of 512)
          ex = exp(S^T/8) on ACT -> SBUF bf16   (the pacing engine)
          PV: per 128-i chunk: o_acc[i,65] += ex_slice^T @ v_aug
              (v_aug has a ones column so col 64 accumulates the
              softmax denominator; M=i layout keeps the PE at
              128 outputs/cycle instead of 65/128 partitions)
        norm: DVE reciprocal of o_acc[:,:,64] + one broadcast multiply
              into O_pair[:,:,h2,:] bf16
      after both heads of (p, ib): 8 DMA-engine transposes
      [128i x (2x64)d] -> OT[:, p, i-slice]  (XBAR dma transpose)
  P2  output projection dripped into PE slack as OT columns become
      available; partial [2048,1024] bf16 DMA'd out per [128,512] tile.

Dripping: v-projection, pair p+1 q/k projection, and output-projection
units are generators yielding one PE matmul per step, pumped into the
attention loop's PE slack (ACT paces the inner loop at ~1038ns/jc vs
~644ns of PE work).
"""

import numpy as np
import ml_dtypes

import concourse.mybir as mybir
import concourse.tile as tile
from concourse import bacc
from concourse.bass_utils import run_bass_kernel_spmd

F32 = mybir.dt.float32
BF16 = mybir.dt.bfloat16
EXP = mybir.ActivationFunctionType.Exp
bfloat16 = ml_dtypes.bfloat16

P = 128
B, N, DIM = 4, 2048, 1024
H_LOC = 8  # heads per core
D = 64
FEAT = H_LOC * D  # 512
KC = DIM // P  # 8 contraction chunks
NT = N // P  # 16 token chunks (j-chunks)
SCALE = 1.0 / 8.0

_CACHE = {}


def _emit(nc, tc):
    from contextlib import ExitStack
    from collections import deque

    xT_d = nc.dram_tensor("xT", [DIM, N], BF16, kind="ExternalInput")
    # wqk pre-shuffled on host: [pair, p, kc, 128] so each pair slice is a
    # single full-speed DMA (2KB contiguous rows)
    wq_d = nc.dram_tensor("wq", [4, P, KC, P], BF16, kind="ExternalInput")
    wk_d = nc.dram_tensor("wk", [4, P, KC, P], BF16, kind="ExternalInput")
    wv_d = nc.dram_tensor("wv", [4, P, KC, P], BF16, kind="ExternalInput")
    wo_d = nc.dram_tensor("wo", [FEAT, DIM], BF16, kind="ExternalInput")
    out_d = nc.dram_tensor("partial", [N, DIM], BF16, kind="ExternalOutput")

    with ExitStack() as ctx:
        big = ctx.enter_context(tc.tile_pool(name="big", bufs=1))

        # ---- persistent SBUF ----
        xT = big.tile([P, KC, N], BF16)  # 32KB/p
        v_aug = big.tile([P, NT, H_LOC, D + 1], BF16)  # 16.25KB/p
        OT = big.tile([P, 4, N], BF16)  # 16KB/p
        wv_sb = big.tile([P, 4, KC, P], BF16)  # 8KB/p
        wo_sb = big.tile([P, 4, DIM], BF16)  # 8KB/p
        wq_sb = big.tile([P, 4, KC, P], BF16)  # 8KB/p
        wk_sb = big.tile([P, 4, KC, P], BF16)  # 8KB/p

        # ones column of v_aug; zeroed warmup operand tile
        nc.vector.memset(v_aug[:, :, :, D], 1.0)
        warm = big.tile([P, 640], BF16)
        nc.vector.memset(warm[:], 0.0)
        ident = big.tile([P, P], BF16)
        from concourse import masks
        masks.make_identity(nc, ident[:])

        # ---- input DMAs (SP queue, in priority order) ----
        # xT in token-halves: the P0 projection (k/q tokens 0-1023) only
        # needs the first half of every kc chunk, so attention starts
        # ~6us after the 8 half-chunks land.
        nc.sync.dma_start(wq_sb[:, 0], wq_d.ap()[0])
        nc.sync.dma_start(wk_sb[:, 0], wk_d.ap()[0])
        for kc in range(KC):
            nc.sync.dma_start(
                xT[:, kc, 0:1024], xT_d.ap()[kc * P : (kc + 1) * P, 0:1024]
            )
        nc.sync.dma_start(wv_sb[:, 0], wv_d.ap()[0])
        for kc in range(KC):
            nc.sync.dma_start(
                xT[:, kc, 1024:N], xT_d.ap()[kc * P : (kc + 1) * P, 1024:N]
            )
        nc.sync.dma_start(wv_sb[:, 1], wv_d.ap()[1])
        nc.sync.dma_start(wq_sb[:, 1], wq_d.ap()[1])
        nc.sync.dma_start(wk_sb[:, 1], wk_d.ap()[1])
        nc.sync.dma_start(wv_sb[:, 2], wv_d.ap()[2])
        nc.sync.dma_start(wv_sb[:, 3], wv_d.ap()[3])
        nc.sync.dma_start(
            wo_sb[:], wo_d.ap().rearrange("(fc p) o -> p fc o", p=P)
        )
        for pr in (2, 3):
            nc.sync.dma_start(wq_sb[:, pr], wq_d.ap()[pr])
            nc.sync.dma_start(wk_sb[:, pr], wk_d.ap()[pr])

        # rotating pools
        qkT = ctx.enter_context(tc.tile_pool(name="qkT", bufs=2))
        exp_pool = ctx.enter_context(tc.tile_pool(name="exp", bufs=26))
        osb = ctx.enter_context(tc.tile_pool(name="osb", bufs=2))
        small = ctx.enter_context(tc.tile_pool(name="small", bufs=2))
        stage = ctx.enter_context(tc.tile_pool(name="stage", bufs=6))

        # ---- P0: the critical quarter of the pair-0 q/k projection ----
        # Only the dsts the first ~8 attention slots need: k tokens 0-1023
        # (j-chunks 0-7) and q tokens 0-1023 (the ib0 i-range).  kc-outer so
        # each xT chunk is consumed as it lands; the other 4 dsts drip into
        # the attention loop.  Warmup matmuls (zeroed operands, discarded
        # results in unused accumulator slots) keep the tensor engine's
        # p-state ramp alive across the DMA-paced stretches.
        DSTS0 = [("k", 0), ("q", 0), ("q", 1), ("k", 1)]
        DSTS1 = [("k", 2), ("q", 2), ("k", 3), ("q", 3)]

        def new_pair_tiles():
            qT = qkT.tile([P, N], BF16, tag="qT")
            kT = qkT.tile([P, N], BF16, tag="kT")
            return qT, kT

        pair_tiles = [None] * 4
        pair_tiles[0] = new_pair_tiles()
        with tc.tile_pool(name="p0ps", bufs=1, space="PSUM") as p0ps:
            acc = p0ps.tile([P, KC, 512], F32)

            def warmup(n):
                for wi in range(n):
                    nc.tensor.matmul(
                        acc[:, 4 + wi % 4], warm[:, 0:P], warm[:, 0:512],
                        start=True, stop=True,
                    )

            warmup(6)
            COPY = mybir.ActivationFunctionType.Copy
            for kc in range(KC):
                for di, (qk, blk) in enumerate(DSTS0):
                    w = wq_sb if qk == "q" else wk_sb
                    nc.tensor.matmul(
                        acc[:, di],
                        w[:, 0, kc],
                        xT[:, kc, blk * 512 : (blk + 1) * 512],
                        start=(kc == 0),
                        stop=(kc == KC - 1),
                    )
                    if kc == KC - 1:
                        dst = (
                            pair_tiles[0][0] if qk == "q" else pair_tiles[0][1]
                        )
                        dsl = dst[:, blk * 512 : (blk + 1) * 512]
                        if di % 2 == 0:
                            # ACT is idle pre-attention: alternate the
                            # copies between ACT and DVE so the first ST's
                            # operands (k0 on ACT, q0 on DVE) land in
                            # parallel rather than serialized on one queue
                            nc.scalar.activation(dsl, acc[:, di], COPY)
                        else:
                            nc.vector.tensor_copy(dsl, acc[:, di])
                if kc < 2:
                    warmup(1)

        # ---- P1 pools ----
        st_pool = ctx.enter_context(
            tc.tile_pool(name="st", bufs=2, space="PSUM")
        )
        oacc_pool = ctx.enter_context(
            tc.tile_pool(name="oacc", bufs=1, space="PSUM")
        )
        aux_pool = ctx.enter_context(
            tc.tile_pool(name="aux", bufs=2, space="PSUM")
        )

        # ---- drip generators (one PE matmul per yield) ----
        v_ready = [0, 0, 0, 0]  # per pair: number of token chunks projected

        def v_unit(pr, tc_i):
            # v for head pair pr (2 heads, 128 feat), token chunk tc_i
            ps = aux_pool.tile([P, P], F32, tag="aux", name=f"vps{pr}_{tc_i}")
            for kc in range(KC):
                nc.tensor.matmul(
                    ps[:],
                    xT[:, kc, tc_i * P : (tc_i + 1) * P],
                    wv_sb[:, pr, kc],
                    start=(kc == 0),
                    stop=(kc == KC - 1),
                )
                yield 53
            nc.vector.tensor_copy(
                v_aug[:, tc_i, 2 * pr : 2 * pr + 2, 0:D],
                ps[:].rearrange("p (h d) -> p h d", d=D),
            )
            v_ready[pr] = tc_i + 1

        proj_done = set()
        for qk, blk in DSTS0:
            proj_done.add((0, qk, blk))

        def proj_unit(pair, qk, blk):
            w = wq_sb if qk == "q" else wk_sb
            dst = pair_tiles[pair][0] if qk == "q" else pair_tiles[pair][1]
            ps = aux_pool.tile([P, 512], F32, tag="aux")
            for kc in range(KC):
                nc.tensor.matmul(
                    ps[:],
                    w[:, pair, kc],
                    xT[:, kc, blk * 512 : (blk + 1) * 512],
                    start=(kc == 0),
                    stop=(kc == KC - 1),
                )
                yield 213
            nc.vector.tensor_copy(dst[:, blk * 512 : (blk + 1) * 512], ps[:])
            proj_done.add((pair, qk, blk))

        def ensure_proj(pair, qk, blk):
            # correctness guard: an ST must never be emitted before the
            # projection unit writing its q/k slice
            while (pair, qk, blk) not in proj_done and fillers:
                pump_one()

        out_r = out_d.ap().rearrange("(tc p) o -> tc p o", p=P)

        def c_unit(tc_i, nb):
            ps = aux_pool.tile([P, 512], F32, tag="aux")
            for fc in range(4):
                nc.tensor.matmul(
                    ps[:],
                    OT[:, fc, tc_i * P : (tc_i + 1) * P],
                    wo_sb[:, fc, nb * 512 : (nb + 1) * 512],
                    start=(fc == 0),
                    stop=(fc == 3),
                )
                yield 213
            st = stage.tile([P, 512], BF16, tag="stg")
            nc.vector.tensor_copy(st[:], ps[:])
            nc.sync.dma_start(out_r[tc_i, :, nb * 512 : (nb + 1) * 512], st[:])

        # late-token units split in two: fc0-2 pre-accumulates into an SBUF
        # partial as soon as pairs 0-2's OT columns exist; only the fc3
        # matmul (gated by the final pair's transposes) plus one DVE add
        # remain on the critical tail
        parts = {}

        def c_pre(tc_i, nb):
            ps = aux_pool.tile([P, 512], F32, tag="aux")
            for fc in range(3):
                nc.tensor.matmul(
                    ps[:],
                    OT[:, fc, tc_i * P : (tc_i + 1) * P],
                    wo_sb[:, fc, nb * 512 : (nb + 1) * 512],
                    start=(fc == 0),
                    stop=(fc == 2),
                )
                yield 213
            prt = stage.tile(
                [P, 512], BF16, tag="part", bufs=16, name=f"prt{tc_i}_{nb}"
            )
            nc.vector.tensor_copy(prt[:], ps[:])
            parts[(tc_i, nb)] = prt

        def c_post(tc_i, nb):
            ps = aux_pool.tile([P, 512], F32, tag="aux")
            nc.tensor.matmul(
                ps[:],
                OT[:, 3, tc_i * P : (tc_i + 1) * P],
                wo_sb[:, 3, nb * 512 : (nb + 1) * 512],
                start=True,
                stop=True,
            )
            yield 213
            st = stage.tile([P, 512], BF16, tag="stg")
            nc.vector.tensor_add(st[:], ps[:], parts[(tc_i, nb)][:])
            nc.sync.dma_start(out_r[tc_i, :, nb * 512 : (nb + 1) * 512], st[:])

        fillers = deque()

        def pump_one():
            # returns the PE cost (ns) of the pumped step
            while fillers:
                try:
                    return next(fillers[0]) or 213
                except StopIteration:
                    fillers.popleft()
            return 0

        def drip(budget_ns):
            while budget_ns > 0 and fillers:
                c = pump_one()
                if c == 0:
                    break
                budget_ns -= c

        def ensure_v(pr, jc):
            # pump fillers until pair pr's v covers token chunk jc; the
            # fillers queue is ordered so this pulls a minimal prefix
            while v_ready[pr] <= jc and fillers:
                pump_one()

        def drain_all():
            while fillers:
                for _ in fillers.popleft():
                    pass

        # ---- attention block ----
        _bid = [0]

        def attn_block(h, i0, iw, o_pair, drip_rate, prev=None, early=True):
            """One head's attention for i in [i0, i0+iw).  iw in {512,1024}.

            Returns a `tail` list of closures (remaining PV emissions +
            the normalization) that the caller either flushes immediately
            or hands to the next block to drain one-per-slot.
            """
            pair, h2 = h // 2, h % 2
            prev = prev or []
            bid = _bid[0] = _bid[0] + 1
            qT, kT = pair_tiles[pair]
            qh = qT[h2 * D : (h2 + 1) * D]
            kh = kT[h2 * D : (h2 + 1) * D]
            nic = iw // P
            o_acc = oacc_pool.tile(
                [P, 8, D + 1], F32, tag="oacc", padded_shape=[P, 8, P]
            )
            exs = [None] * NT

            def pv_half(ic, lo, hi):
                # accumulation groups must never interleave within one PSUM
                # bank: a column's group opens at jc=0 and closes at jc=15,
                # and columns are emitted strictly one after another (the
                # second half of column k always precedes the first half of
                # column k+1)
                ensure_v(pair, hi - 1)
                for jc in range(lo, hi):
                    nc.tensor.matmul(
                        o_acc[:, ic, :],
                        exs[jc][:, ic * P : (ic + 1) * P],
                        v_aug[:, jc, h],
                        start=(jc == 0),
                        stop=(jc == NT - 1),
                    )

            def pv_col(ic):
                pv_half(ic, 0, NT)

            def emit_norm():
                recip = small.tile([P, 8], F32, tag="recip")
                nc.vector.reciprocal(recip[:, 0:nic], o_acc[:, 0:nic, D])
                s0 = (i0 % 1024) // P
                nc.vector.tensor_mul(
                    o_pair[:, s0 : s0 + nic, h2, :],
                    o_acc[:, 0:nic, 0:D],
                    recip[:, 0:nic]
                    .rearrange("p (a b) -> p a b", b=1)
                    .to_broadcast([P, nic, D]),
                )

            col_cost = 16 * 65 * 0.4167  # ns per PV column
            for jc in range(NT):
                ensure_proj(pair, "k", jc // 4)
                for hf0 in range(iw // 512):
                    ensure_proj(pair, "q", i0 // 512 + hf0)
                st = st_pool.tile([P, iw], F32, tag="st")
                for hf in range(iw // 512):
                    nc.tensor.matmul(
                        st[:, hf * 512 : (hf + 1) * 512],
                        kh[:, jc * P : (jc + 1) * P],
                        qh[:, i0 + hf * 512 : i0 + (hf + 1) * 512],
                        start=True,
                        stop=True,
                    )
                ex = exp_pool.tile([P, iw], BF16, tag="ex")
                nc.scalar.activation(ex[:], st[:], EXP, scale=SCALE)
                exs[jc] = ex
                # drain earlier blocks' tails with cost-aware drip top-up so
                # zero-PE items (norms, transpose triggers) don't starve PE
                budget = drip_rate(jc)
                if prev and (len(prev) >= 5 or jc % 2 == 0):
                    item = prev.pop(0)
                    item[1]()
                    budget -= item[2]
                drip(max(60, budget))
            tail = [
                (bid, (lambda ic=ic: pv_col(ic)), col_cost)
                for ic in range(nic)
            ]
            tail.append((bid, emit_norm, 0))
            return (prev or []) + tail

        def transposes(pair, ib, o_pair, ic_range, on_pe=False):
            for ic in ic_range:
                dst = OT[:, pair, ib * 1024 + ic * P : ib * 1024 + (ic + 1) * P]
                src = o_pair[:, ic].rearrange("p a b -> p (a b)")
                if on_pe:
                    # low-latency path for the final blocks: PE transpose +
                    # DVE copy instead of the ~3.5us XBAR DMA chain
                    tp = aux_pool.tile([P, P], BF16, tag="aux", name=f"tp{ic}")
                    nc.tensor.transpose(tp[:], src, ident[:])
                    nc.vector.tensor_copy(dst, tp[:])
                else:
                    nc.sync.dma_start_transpose(dst, src)

        # ---- main loop ----
        # drip rates: heavy during h0 (v-projection), medium h1 (pair-1
        # proj), light elsewhere
        def mk_rate(base_ns, first_extra_ns=0):
            def rate(jc):
                return base_ns + (first_extra_ns if jc < 2 else 0)
            return rate

        carry = [[], None, 0]  # [tail, deferred ib1-transposes, their bid]
        for pair in range(4):
            if pair == 0:
                # pair-0 v interleaved with the deferred quarter of the
                # pair-0 q/k projection, ordered so ensure_v pulls a
                # minimal prefix while the deferred dsts land before the
                # ST j/i ranges that need them (k2 by jc8, k3 by jc12,
                # q2/q3 by ib1)
                defer = {2: proj_unit(0, "k", 2), 5: proj_unit(0, "k", 3),
                         7: proj_unit(0, "q", 2), 9: proj_unit(0, "q", 3)}
                for tc_i in range(NT):
                    fillers.append(v_unit(0, tc_i))
                    if tc_i in defer:
                        fillers.append(defer[tc_i])
            if pair < 3:
                pair_tiles[pair + 1] = new_pair_tiles()
                for qk, blk in (("q", 0), ("k", 0), ("q", 1), ("k", 1),
                                ("q", 2), ("k", 2), ("q", 3), ("k", 3)):
                    fillers.append(proj_unit(pair + 1, qk, blk))
                for tc_i in range(NT):
                    fillers.append(v_unit(pair + 1, tc_i))
            h_lo, h_hi = 2 * pair, 2 * pair + 1
            o_pairs = [
                osb.tile([P, 8, 2, D], BF16, tag="opair", name=f"op{pair}_{i}")
                for i in range(2)
            ]
            if pair == 0:
                rate = mk_rate(640, 300)
            elif pair == 3:
                rate = mk_rate(450, 350)
            else:
                rate = mk_rate(340, 350)
            last_pair = pair == 3

            def flush(tail, upto=None):
                while tail and (upto is None or tail[0][0] <= upto):
                    tail.pop(0)[1]()
                return tail

            if not last_pair:
                t = attn_block(h_lo, 0, 1024, o_pairs[0], rate, prev=carry[0],
                               early=(pair > 0))
                if carry[1] is not None:
                    t = flush(t, upto=carry[2])
                    carry[1]()
                t = attn_block(h_hi, 0, 1024, o_pairs[0], rate, prev=t)
                bid_b = _bid[0]
                t = attn_block(h_lo, 1024, 1024, o_pairs[1], rate, prev=t)
                t = flush(t, upto=bid_b)
                transposes(pair, 0, o_pairs[0], range(8))
                t = attn_block(h_hi, 1024, 1024, o_pairs[1], rate, prev=t)
                carry = [
                    t,
                    (lambda pr=pair, op=o_pairs[1]:
                        transposes(pr, 1, op, range(8))),
                    _bid[0],
                ]
            else:
                def tp_ib0():
                    transposes(pair, 0, o_pairs[0], range(8))
                    # tokens 0..1023 complete across all pairs
                    for tc_i in range(8):
                        for nb in range(2):
                            fillers.append(c_unit(tc_i, nb))

                def tp_ib1a():
                    transposes(pair, 1, o_pairs[1], range(4), on_pe=True)
                    for tc_i in range(8, 12):
                        for nb in range(2):
                            fillers.append(c_post(tc_i, nb))

                t = attn_block(h_lo, 0, 1024, o_pairs[0], rate, prev=carry[0])
                if carry[1] is not None:
                    t = flush(t, upto=carry[2])
                    carry[1]()
                for tc_i in range(8, 16):
                    for nb in range(2):
                        fillers.append(c_pre(tc_i, nb))
                t = attn_block(h_hi, 0, 1024, o_pairs[0], rate, prev=t)
                t.append((_bid[0], tp_ib0, 0))
                # split the final head's ib1 to shrink the tail
                t = attn_block(h_lo, 1024, 1024, o_pairs[1], rate, prev=t)
                t = attn_block(h_hi, 1024, 512, o_pairs[1], rate, prev=t)
                t.append((_bid[0], tp_ib1a, 0))
                t = attn_block(h_hi, 1536, 512, o_pairs[1], rate, prev=t)
                flush(t)
                transposes(pair, 1, o_pairs[1], range(4, 8), on_pe=True)
                for tc_i in range(12, 16):
                    for nb in range(2):
                        fillers.append(c_post(tc_i, nb))
        drain_all()


def _build():
    nc = bacc.Bacc("TRN2", target_bir_lowering=False, debug=False)
    with nc.allow_low_precision(reason="bf16 attention within tolerance"):
        with tile.TileContext(nc) as tc:
            _emit(nc, tc)
    nc.compile()
    return nc


def _get_nc():
    if "nc" not in _CACHE:
        _CACHE["nc"] = _build()
    return _CACHE["nc"]


def kernel(x, w_qkv, w_out, b_out, _trace=False, _tmpdir=None):
    x = np.asarray(x, dtype=np.float32)
    w_qkv = np.asarray(w_qkv, dtype=np.float32)
    w_out = np.asarray(w_out, dtype=np.float32)
    b_out = np.asarray(b_out, dtype=np.float32)

    nc = _get_nc()
    in_maps = []
    for j in range(8):
        b, hg = j // 2, j % 2
        s = FEAT * hg
        wq = w_qkv[:, s : s + FEAT]
        wk = w_qkv[:, DIM + s : DIM + s + FEAT]
        wv = w_qkv[:, 2 * DIM + s : 2 * DIM + s + FEAT]
        # pre-shuffle wq/wk into [pair, p, kc, 128]
        shuf = lambda w: np.ascontiguousarray(
            w.reshape(KC, P, 4, P).transpose(2, 1, 0, 3)
        ).astype(bfloat16)
        in_maps.append(
            {
                "xT": np.ascontiguousarray(x[b].T).astype(bfloat16),
                "wq": shuf(wq),
                "wk": shuf(wk),
                "wv": shuf(wv),
                "wo": np.ascontiguousarray(w_out[s : s + FEAT, :]).astype(
                    bfloat16
                ),
            }
        )
    res = run_bass_kernel_spmd(
        nc, in_maps, core_ids=list(range(8)), trace=_trace, tmpdir=_tmpdir
    )
    out = np.empty((B, N, DIM), np.float32)
    for b in range(B):
        out[b] = res.results[2 * b]["partial"].astype(np.float32)
        out[b] += res.results[2 * b + 1]["partial"].astype(np.float32)
    out += b_out[None, None, :]
    if _trace:
        return out, res
    return out
